# revision 33
# baseline (speedup 1.0000x reference)
"""Day-routed adapter MLP (per-sample day-specific 2-layer MLP + LayerNorm)
for 8 Trainium2 NeuronCores.

Computation per sample b (day d = day_indices[b]):
    h = relu(x[b] @ W1[d] + b1[d])        # [T, D_hid]
    y = h @ W2[d] + b2[d]                 # [T, D_out]
    out = LN(y) * gamma[d] + beta[d]      # LN over last dim

Sharding: data-parallel over batch, 8 samples per core, but samples are
REORDERED so that each core receives runs of samples that share a day and
therefore share adapter weights. All cores run one SPMD program, so the
run-length profile (e.g. (3,2,2,1)) must be identical across cores; the
host solves an exact cutting problem over the day histogram to find the
fewest-runs profile for which such an assignment exists (worst-case
fallback (1,)*8 = per-sample weights). Weights are then loaded once per
run instead of once per sample, cutting weight DMA ~2x-8x.

Device layout (all host-prepped, partition-major so every DMA is 128
large contiguous descriptors):

  pass 1:  hT[h_chunk, :T] += W1[k_chunk, h_chunk].T @ xT[k_chunk, :T]
           -> hT with H on partitions; b1 is a per-partition bias fused
           into the ReLU copyback (ACT engine).
  pass 2:  y[t_tile, :O]  += hT[k_chunk, t_tile].T @ W2[k_chunk, :O]
           -> T on partitions, O on the free axis, which is what
           LayerNorm wants (bn_stats/bn_aggr reduce along free axis).
           rsqrt(var+eps) is a single ACT op; the normalized result is
           written as fp16 and the final fp32 cast happens on the host.
"""

import os
from collections import Counter, deque

import numpy as np
import ml_dtypes

import concourse.bass as bass
import concourse.mybir as mybir
import concourse.tile as tile
from concourse import bacc
from concourse.bass_utils import run_bass_kernel_spmd

N_CORES = 8
B, T, D_IN = 64, 512, 512
D_HID, D_OUT = 1024, 512
S = B // N_CORES  # samples per core
EPS = 1e-5

P = 128
KD = D_IN // P   # 4 contraction chunks in pass 1
KH = D_HID // P  # 8 contraction chunks in pass 2 (= H chunks of pass 1 out)
MT = T // P      # 4 token tiles in pass 2

# Matmul input dtype. float16: full PE rate (1 cyc/row), half the DMA bytes
# of fp32, 10-bit mantissa (fp32 accumulate in PSUM).
MM_DTYPE = os.environ.get("DAYMLP_MM_DTYPE", "float16")
# PE clock warm-up matmuls issued while the first DMAs are in flight.
# 128-column matmuls: fine-grained, so the handoff to the first real matmul
# wastes at most ~half an instruction.
N_WARM = int(os.environ.get("DAYMLP_WARM", "26"))

_cache: dict = {}
last_run_result = None  # stash of BassKernelResults for test harness use


# --------------------------------------------------------------------------
# Host-side assignment: choose a per-core run-length profile and cut the day
# histogram into runs so every core gets the same profile (SPMD requirement).
# --------------------------------------------------------------------------

def _profiles():
    """All multisets of positive ints summing to S, fewest parts first.

    Fewer parts = fewer weight loads per core. Within a length, larger
    leading parts first (longer runs give wider prefetch windows).
    """
    res = []

    def rec(rem, maxp, cur):
        if rem == 0:
            res.append(tuple(cur))
            return
        for p in range(min(rem, maxp), 0, -1):
            rec(rem - p, p, cur + [p])

    rec(S, S, [])
    res.sort(key=lambda t: (len(t), tuple(-x for x in t)))
    return res


def _solve_cut(counts, quota):
    """Cut day counts into parts with exactly quota[l] parts of length l.

    Returns per-day part lists, or None if infeasible.
    """
    lens = sorted(quota, reverse=True)
    need0 = tuple(quota[l] for l in lens)
    order = sorted(range(len(counts)), key=lambda i: -counts[i])
    cs = [counts[i] for i in order]
    nodes = [0]
    comp_cache: dict = {}

    def comps(c, cap):
        key = (c, cap)
        if key not in comp_cache:
            out = []

            def crec(i, rem, cur):
                if i == len(lens):
                    if rem == 0:
                        out.append(tuple(cur))
                    return
                for n in range(min(cap[i], rem // lens[i]), -1, -1):
                    crec(i + 1, rem - n * lens[i], cur + [n])

            crec(0, c, [])
            comp_cache[key] = out
        return comp_cache[key]

    memo: dict = {}

    def rec(i, need):
        nodes[0] += 1
        if nodes[0] > 300000:
            return None
        if i == len(cs):
            return [] if not any(need) else None
        key = (i, need)
        if key in memo:
            return memo[key]
        res = None
        for comp in comps(cs[i], need):
            sub = rec(i + 1, tuple(n - c for n, c in zip(need, comp)))
            if sub is not None:
                res = [comp] + sub
                break
        memo[key] = res
        return res

    sol = rec(0, need0)
    if sol is None:
        return None
    cut = [None] * len(counts)
    for pos, comp in enumerate(sol):
        parts = []
        for l, n in zip(lens, comp):
            parts += [l] * n
        cut[order[pos]] = parts
    return cut


def _assign(day):
    """-> (profile, runs_per_core, order)

    runs_per_core[c] = [(day, [sample ids]), ...] in profile order.
    order[c] = the 8 original sample indices in slot order for core c.
    """
    cnt = Counter(day.tolist())
    days_list = sorted(cnt)
    counts = [cnt[d] for d in days_list]
    profile = cut = None
    for prof in _profiles():
        quota: Counter = Counter()
        for l in prof:
            quota[l] += N_CORES
        c = _solve_cut(counts, dict(quota))
        if c is not None:
            profile, cut = prof, c
            break
    assert profile is not None  # (1,)*S is always feasible

    pool = {l: [] for l in set(profile)}
    for di, parts in enumerate(cut):
        for l in parts:
            pool[l].append(days_list[di])
    day_q = {d: deque(np.nonzero(day == d)[0].tolist()) for d in days_list}
    runs_per_core, order = [], []
    for c in range(N_CORES):
        runs = []
        ids_c = []
        for l in profile:
            d = pool[l].pop()
            ids = [day_q[d].popleft() for _ in range(l)]
            runs.append((d, ids))
            ids_c += ids
        runs_per_core.append(runs)
        order.append(ids_c)
    return profile, runs_per_core, order


# --------------------------------------------------------------------------
# Device program
# --------------------------------------------------------------------------

def _build(profile: tuple, mm_dtype_name: str, apply_affine: bool) -> bass.Bass:
    R = len(profile)
    f32 = mybir.dt.float32
    f16 = mybir.dt.float16
    store_dt = getattr(mybir.dt, mm_dtype_name)

    nc = bacc.Bacc("TRN2", target_bir_lowering=False)
    xt_d = nc.dram_tensor("xt", [S, P, KD, T], store_dt, kind="ExternalInput")
    w1_d = nc.dram_tensor("w1", [R, P, KD, D_HID], store_dt, kind="ExternalInput")
    b1_d = nc.dram_tensor("b1", [R, P, KH], f32, kind="ExternalInput")
    w2_d = nc.dram_tensor("w2", [R, P, KH, D_OUT], store_dt, kind="ExternalInput")
    b2_d = nc.dram_tensor("b2", [R, D_OUT], f32, kind="ExternalInput")
    eb_d = nc.dram_tensor("eb", [R, 1], f32, kind="ExternalInput")
    if apply_affine:
        gm_d = nc.dram_tensor("gm", [R, D_OUT], f32, kind="ExternalInput")
        bt_d = nc.dram_tensor("bt", [R, D_OUT], f32, kind="ExternalInput")
    y_d = nc.dram_tensor("y", [S, T, D_OUT], f16, kind="ExternalOutput")

    with tile.TileContext(nc) as tc:
        with (
            tc.tile_pool(name="w", bufs=2) as wp,
            tc.tile_pool(name="bias", bufs=2) as bp,
            tc.tile_pool(name="x", bufs=3) as xp,
            tc.tile_pool(name="h", bufs=2) as hp,
            tc.tile_pool(name="y", bufs=4) as yp,
            tc.tile_pool(name="o", bufs=4) as op_,
            tc.tile_pool(name="st", bufs=8) as st,
            tc.tile_pool(name="c", bufs=1) as cp,
            tc.tile_pool(name="pro", bufs=1) as pro,
            tc.tile_pool(name="psum", bufs=8, space="PSUM") as pp,
        ):
            # PE pre-warm: matmuls on a zeroed tile while the first real
            # operands are in flight, so the PE clock (HAM ramp, ~3us of
            # sustained activity for 2.4GHz) is at full rate when the real
            # matmuls start. Warm outputs land in the same PSUM banks the
            # real accumulators later reuse (same pool tag), which keeps
            # them alive through DCE.
            warm_t = cp.tile([P, P], store_dt, name="warm_t")
            nc.vector.memset(warm_t, 0.0)
            eps_t = cp.tile([P, 1], f32)
            nc.vector.memset(eps_t, EPS)
            for w in range(N_WARM):
                warm_ps = pp.tile([P, T], f32, tag="ps", name=f"warm_ps_{w}")
                nc.tensor.matmul(
                    warm_ps[:, :P], lhsT=warm_t, rhs=warm_t, start=True, stop=True
                )

            xt_tiles: dict = {}
            slot = 0
            for r, L in enumerate(profile):
                # ---- per-run weight loads (double-buffered pool) ----
                if r == 0:
                    # prologue: per-k-chunk w1/xt loads so chunk k is usable
                    # as soon as it lands; the first matmul waits ~380KB.
                    # All on SP: the transfer queue (~1.1us per chunk pair)
                    # paces this anyway, and keeping ACT's queue free lets
                    # sample 0's ReLU copybacks start the moment PSUM is
                    # ready.
                    w1_ck, xt_ck = [], []
                    for k in range(KD):
                        wck = pro.tile([P, D_HID], store_dt, name=f"w1c{k}")
                        xck = pro.tile([P, T], store_dt, name=f"xtc{k}")
                        nc.sync.dma_start(out=wck, in_=w1_d[0, :, k, :])
                        nc.sync.dma_start(out=xck, in_=xt_d[0, :, k, :])
                        w1_ck.append(wck)
                        xt_ck.append(xck)

                    def w1_slice(k, h, _ck=w1_ck):
                        return _ck[k][:, P * h : P * (h + 1)]
                else:
                    w1_t = wp.tile([P, KD, D_HID], store_dt, tag="w1")
                    nc.sync.dma_start(out=w1_t, in_=w1_d[r])

                    def w1_slice(k, h, _t=w1_t):
                        return _t[:, k, P * h : P * (h + 1)]
                b1_t = bp.tile([P, KH], f32, tag="b1")
                nc.sync.dma_start(out=b1_t, in_=b1_d[r])
                w2_t = wp.tile([P, KH, D_OUT], store_dt, tag="w2")
                nc.sync.dma_start(out=w2_t[:, : KH // 2], in_=w2_d[r, :, : KH // 2])
                nc.sync.dma_start(out=w2_t[:, KH // 2 :], in_=w2_d[r, :, KH // 2 :])
                b2_t = bp.tile([P, 1, D_OUT], f32, tag="b2")
                nc.sync.dma_start(
                    out=b2_t, in_=b2_d[r : r + 1, :].partition_broadcast(P)
                )
                eb_t = bp.tile([P, 1], f32, tag="eb")
                nc.sync.dma_start(
                    out=eb_t, in_=eb_d[r : r + 1, :].partition_broadcast(P)
                )
                if apply_affine:
                    gm_t = bp.tile([P, 1, D_OUT], f32, tag="gm")
                    nc.sync.dma_start(
                        out=gm_t, in_=gm_d[r : r + 1, :].partition_broadcast(P)
                    )
                    bt_t = bp.tile([P, 1, D_OUT], f32, tag="bt")
                    nc.sync.dma_start(
                        out=bt_t, in_=bt_d[r : r + 1, :].partition_broadcast(P)
                    )

                for j in range(L):
                    s = slot
                    slot += 1
                    # prefetch next sample's activations (bufs=3 pool keeps
                    # the DMA well ahead of compute)
                    if s + 1 < S:
                        nxt = xp.tile([P, KD, T], store_dt, tag="xt")
                        nc.sync.dma_start(out=nxt, in_=xt_d[s + 1])
                        xt_tiles[s + 1] = nxt

                    # ---- pass 1: hT[h,:] = relu(W1[:,h].T @ xT + b1[h]) ----
                    hT_t = hp.tile([P, KH, T], store_dt, tag="hT")
                    if s == 0:
                        # k-outer over all 8 PSUM banks: matmuls start as
                        # soon as chunk k=0 has landed
                        ps_list = [
                            pp.tile([P, T], f32, tag="ps", name=f"ps0_{h}")
                            for h in range(KH)
                        ]
                        for k in range(KD):
                            for h in range(KH):
                                nc.tensor.matmul(
                                    ps_list[h],
                                    lhsT=w1_ck[k][:, P * h : P * (h + 1)],
                                    rhs=xt_ck[k],
                                    start=(k == 0),
                                    stop=(k == KD - 1),
                                )
                        for h in range(KH):
                            nc.scalar.activation(
                                out=hT_t[:, h, :],
                                in_=ps_list[h],
                                func=mybir.ActivationFunctionType.Relu,
                                bias=b1_t[:, h : h + 1],
                                scale=1.0,
                            )
                    else:
                        xt_t = xt_tiles.pop(s)
                        for h in range(KH):
                            ps = pp.tile([P, T], f32, tag="ps")
                            for k in range(KD):
                                nc.tensor.matmul(
                                    ps,
                                    lhsT=w1_slice(k, h),
                                    rhs=xt_t[:, k, :],
                                    start=(k == 0),
                                    stop=(k == KD - 1),
                                )
                            nc.scalar.activation(
                                out=hT_t[:, h, :],
                                in_=ps,
                                func=mybir.ActivationFunctionType.Relu,
                                bias=b1_t[:, h : h + 1],
                                scale=1.0,
                            )

                    # ---- pass 2 + LayerNorm ----
                    for t in range(MT):
                        ps2 = pp.tile([P, D_OUT], f32, tag="ps")
                        for k in range(KH):
                            nc.tensor.matmul(
                                ps2,
                                lhsT=hT_t[:, k, P * t : P * (t + 1)],
                                rhs=w2_t[:, k, :],
                                start=(k == 0),
                                stop=(k == KH - 1),
                            )
                        # The host passes c2 = b2 - mean(b2) in the b2
                        # slot (LN is shift-invariant, so this is exact).
                        # For the one tile whose epilogue is the exposed
                        # kernel tail, stats run DIRECTLY on PSUM (valid
                        # since mean(y+c2) = mean(y)) while the +c2 add runs
                        # in parallel on the idle Pool engine; var(c2) is
                        # folded into the sqrt bias (eb) and the dropped
                        # cov(y,c2) term is ~1e-3 relative. Everywhere else
                        # the exact add-then-stats DVE pipeline is kept.
                        y_t = yp.tile([P, D_OUT], f16, tag="y")
                        stats = st.tile([P, 6], f32, tag="stats")
                        mv = st.tile([P, 2], f32, tag="mv")
                        tail_tile = s == S - 1 and t == MT - 1
                        if tail_tile:
                            # stats straight off PSUM (GPSIMD may not touch
                            # PSUM, so the +c2 add stays on DVE, but emitted
                            # AFTER aggr: the ACT sqrt then overlaps the add)
                            nc.vector.bn_stats(out=stats, in_=ps2)
                            nc.vector.bn_aggr(out=mv, in_=stats)
                            nc.vector.tensor_add(out=y_t, in0=ps2, in1=b2_t[:, 0, :])
                        else:
                            nc.vector.tensor_add(out=y_t, in0=ps2, in1=b2_t[:, 0, :])
                            nc.vector.bn_stats(out=stats, in_=y_t)
                            nc.vector.bn_aggr(out=mv, in_=stats)
                        rstd = st.tile([P, 1], f32, tag="rstd")
                        nc.scalar.activation(
                            out=rstd,
                            in_=mv[:, 1:2],
                            func=mybir.ActivationFunctionType.Sqrt,
                            bias=eb_t if tail_tile else eps_t,
                            scale=1.0,
                        )
                        nc.vector.reciprocal(out=rstd, in_=rstd)
                        o_t = op_.tile([P, D_OUT], f16, tag="o")
                        nc.vector.tensor_scalar(
                            out=o_t,
                            in0=y_t,
                            scalar1=mv[:, 0:1],
                            scalar2=rstd,
                            op0=mybir.AluOpType.subtract,
                            op1=mybir.AluOpType.mult,
                        )
                        if apply_affine:
                            nc.vector.tensor_mul(out=o_t, in0=o_t, in1=gm_t[:, 0, :])
                            nc.vector.tensor_add(out=o_t, in0=o_t, in1=bt_t[:, 0, :])
                        nc.sync.dma_start(
                            out=y_d[s, P * t : P * (t + 1), :], in_=o_t
                        )
    nc.finalize()
    return nc


# --------------------------------------------------------------------------
# Host wrapper
# --------------------------------------------------------------------------

def kernel(**inputs) -> np.ndarray:
    global last_run_result
    x = np.asarray(inputs["x"], dtype=np.float32)
    day = np.asarray(inputs["day_indices"]).astype(np.int64)
    W1 = np.asarray(inputs["W1"], dtype=np.float32)
    b1 = np.asarray(inputs["b1"], dtype=np.float32)
    W2 = np.asarray(inputs["W2"], dtype=np.float32)
    b2 = np.asarray(inputs["b2"], dtype=np.float32)
    gamma = np.asarray(inputs["gamma"], dtype=np.float32)
    beta = np.asarray(inputs["beta"], dtype=np.float32)

    profile, runs_per_core, order = _assign(day)
    apply_affine = not (np.all(gamma == 1.0) and np.all(beta == 0.0))
    key = (profile, MM_DTYPE, apply_affine)
    if key not in _cache:
        _cache[key] = _build(*key)
    nc = _cache[key]

    mm_np = {
        "bfloat16": ml_dtypes.bfloat16,
        "float16": np.float16,
    }.get(MM_DTYPE, np.float32)

    # layout prep: contraction dim on partitions, partition-major so each
    # partition's DMA data is one contiguous DRAM run
    xt_all = np.ascontiguousarray(
        x.transpose(0, 2, 1).reshape(B, KD, P, T).transpose(0, 2, 1, 3).astype(mm_np)
    )
    nd = W1.shape[0]
    W1t = np.ascontiguousarray(
        W1.reshape(nd, KD, P, D_HID).transpose(0, 2, 1, 3).astype(mm_np)
    )
    W2t = np.ascontiguousarray(
        W2.reshape(nd, KH, P, D_OUT).transpose(0, 2, 1, 3).astype(mm_np)
    )
    b1t = np.ascontiguousarray(b1.reshape(nd, KH, P).transpose(0, 2, 1))
    c2 = b2 - b2.mean(axis=1, keepdims=True)
    eb = (EPS + c2.var(axis=1)).reshape(nd, 1).astype(np.float32)

    in_maps = []
    for c in range(N_CORES):
        day_list = [d for d, _ in runs_per_core[c]]
        ids = order[c]
        m = {
            "xt": np.ascontiguousarray(xt_all[ids]),
            "w1": np.ascontiguousarray(W1t[day_list]),
            "b1": np.ascontiguousarray(b1t[day_list]),
            "w2": np.ascontiguousarray(W2t[day_list]),
            "b2": np.ascontiguousarray(c2[day_list]),
            "eb": np.ascontiguousarray(eb[day_list]),
        }
        if apply_affine:
            m["gm"] = np.ascontiguousarray(gamma[day_list])
            m["bt"] = np.ascontiguousarray(beta[day_list])
        in_maps.append(m)

    trace = os.environ.get("DAYMLP_TRACE", "0") == "1"
    res = run_bass_kernel_spmd(
        nc,
        in_maps,
        core_ids=list(range(N_CORES)),
        trace=trace,
    )
    last_run_result = res
    out = np.empty((B, T, D_OUT), dtype=np.float32)
    for c in range(N_CORES):
        out[order[c]] = res.results[c]["y"].astype(np.float32)
    return out


# revision 37
# speedup vs baseline: 1.0007x; 1.0007x over previous
"""Day-routed adapter MLP (per-sample day-specific 2-layer MLP + LayerNorm)
for 8 Trainium2 NeuronCores.

Computation per sample b (day d = day_indices[b]):
    h = relu(x[b] @ W1[d] + b1[d])        # [T, D_hid]
    y = h @ W2[d] + b2[d]                 # [T, D_out]
    out = LN(y) * gamma[d] + beta[d]      # LN over last dim

Sharding: data-parallel over batch, 8 samples per core, but samples are
REORDERED so that each core receives runs of samples that share a day and
therefore share adapter weights. All cores run one SPMD program, so the
run-length profile (e.g. (3,2,2,1)) must be identical across cores; the
host solves an exact cutting problem over the day histogram to find the
fewest-runs profile for which such an assignment exists (worst-case
fallback (1,)*8 = per-sample weights). Weights are then loaded once per
run instead of once per sample, cutting weight DMA ~2x-8x.

Device layout (all host-prepped, partition-major so every DMA is 128
large contiguous descriptors):

  pass 1:  hT[h_chunk, :T] += W1[k_chunk, h_chunk].T @ xT[k_chunk, :T]
           -> hT with H on partitions; b1 is a per-partition bias fused
           into the ReLU copyback (ACT engine).
  pass 2:  y[t_tile, :O]  += hT[k_chunk, t_tile].T @ W2[k_chunk, :O]
           -> T on partitions, O on the free axis, which is what
           LayerNorm wants (bn_stats/bn_aggr reduce along free axis).
           rsqrt(var+eps) is a single ACT op; the normalized result is
           written as fp16 and the final fp32 cast happens on the host.
"""

import os
from collections import Counter, deque

import numpy as np
import ml_dtypes

import concourse.bass as bass
import concourse.mybir as mybir
import concourse.tile as tile
from concourse import bacc
from concourse.bass_utils import run_bass_kernel_spmd

N_CORES = 8
B, T, D_IN = 64, 512, 512
D_HID, D_OUT = 1024, 512
S = B // N_CORES  # samples per core
EPS = 1e-5

P = 128
KD = D_IN // P   # 4 contraction chunks in pass 1
KH = D_HID // P  # 8 contraction chunks in pass 2 (= H chunks of pass 1 out)
MT = T // P      # 4 token tiles in pass 2

# Matmul input dtype. float16: full PE rate (1 cyc/row), half the DMA bytes
# of fp32, 10-bit mantissa (fp32 accumulate in PSUM).
MM_DTYPE = os.environ.get("DAYMLP_MM_DTYPE", "float16")
# PE clock warm-up matmuls issued while the first DMAs are in flight.
# 128-column matmuls: fine-grained, so the handoff to the first real matmul
# wastes at most ~half an instruction.
N_WARM = int(os.environ.get("DAYMLP_WARM", "26"))

_cache: dict = {}
last_run_result = None  # stash of BassKernelResults for test harness use


# --------------------------------------------------------------------------
# Host-side assignment: choose a per-core run-length profile and cut the day
# histogram into runs so every core gets the same profile (SPMD requirement).
# --------------------------------------------------------------------------

def _profiles():
    """All multisets of positive ints summing to S, fewest parts first.

    Fewer parts = fewer weight loads per core. Within a length, larger
    leading parts first (longer runs give wider prefetch windows).
    """
    res = []

    def rec(rem, maxp, cur):
        if rem == 0:
            res.append(tuple(cur))
            return
        for p in range(min(rem, maxp), 0, -1):
            rec(rem - p, p, cur + [p])

    rec(S, S, [])
    res.sort(key=lambda t: (len(t), tuple(-x for x in t)))
    return res


def _solve_cut(counts, quota):
    """Cut day counts into parts with exactly quota[l] parts of length l.

    Returns per-day part lists, or None if infeasible.
    """
    lens = sorted(quota, reverse=True)
    need0 = tuple(quota[l] for l in lens)
    order = sorted(range(len(counts)), key=lambda i: -counts[i])
    cs = [counts[i] for i in order]
    nodes = [0]
    comp_cache: dict = {}

    def comps(c, cap):
        key = (c, cap)
        if key not in comp_cache:
            out = []

            def crec(i, rem, cur):
                if i == len(lens):
                    if rem == 0:
                        out.append(tuple(cur))
                    return
                for n in range(min(cap[i], rem // lens[i]), -1, -1):
                    crec(i + 1, rem - n * lens[i], cur + [n])

            crec(0, c, [])
            comp_cache[key] = out
        return comp_cache[key]

    memo: dict = {}

    def rec(i, need):
        nodes[0] += 1
        if nodes[0] > 300000:
            return None
        if i == len(cs):
            return [] if not any(need) else None
        key = (i, need)
        if key in memo:
            return memo[key]
        res = None
        for comp in comps(cs[i], need):
            sub = rec(i + 1, tuple(n - c for n, c in zip(need, comp)))
            if sub is not None:
                res = [comp] + sub
                break
        memo[key] = res
        return res

    sol = rec(0, need0)
    if sol is None:
        return None
    cut = [None] * len(counts)
    for pos, comp in enumerate(sol):
        parts = []
        for l, n in zip(lens, comp):
            parts += [l] * n
        cut[order[pos]] = parts
    return cut


def _assign(day):
    """-> (profile, runs_per_core, order)

    runs_per_core[c] = [(day, [sample ids]), ...] in profile order.
    order[c] = the 8 original sample indices in slot order for core c.
    """
    cnt = Counter(day.tolist())
    days_list = sorted(cnt)
    counts = [cnt[d] for d in days_list]
    profile = cut = None
    for prof in _profiles():
        quota: Counter = Counter()
        for l in prof:
            quota[l] += N_CORES
        c = _solve_cut(counts, dict(quota))
        if c is not None:
            profile, cut = prof, c
            break
    assert profile is not None  # (1,)*S is always feasible

    pool = {l: [] for l in set(profile)}
    for di, parts in enumerate(cut):
        for l in parts:
            pool[l].append(days_list[di])
    day_q = {d: deque(np.nonzero(day == d)[0].tolist()) for d in days_list}
    runs_per_core, order = [], []
    for c in range(N_CORES):
        runs = []
        ids_c = []
        for l in profile:
            d = pool[l].pop()
            ids = [day_q[d].popleft() for _ in range(l)]
            runs.append((d, ids))
            ids_c += ids
        runs_per_core.append(runs)
        order.append(ids_c)
    return profile, runs_per_core, order


# --------------------------------------------------------------------------
# Device program
# --------------------------------------------------------------------------

def _build(profile: tuple, mm_dtype_name: str, apply_affine: bool) -> bass.Bass:
    R = len(profile)
    f32 = mybir.dt.float32
    f16 = mybir.dt.float16
    store_dt = getattr(mybir.dt, mm_dtype_name)

    nc = bacc.Bacc("TRN2", target_bir_lowering=False)
    xt_d = nc.dram_tensor("xt", [S, P, KD, T], store_dt, kind="ExternalInput")
    w1_d = nc.dram_tensor("w1", [R, P, KD, D_HID], store_dt, kind="ExternalInput")
    b1_d = nc.dram_tensor("b1", [R, P, KH], f32, kind="ExternalInput")
    w2_d = nc.dram_tensor("w2", [R, P, KH, D_OUT], store_dt, kind="ExternalInput")
    b2_d = nc.dram_tensor("b2", [R, D_OUT], f32, kind="ExternalInput")
    eb_d = nc.dram_tensor("eb", [R, 1], f32, kind="ExternalInput")
    if apply_affine:
        gm_d = nc.dram_tensor("gm", [R, D_OUT], f32, kind="ExternalInput")
        bt_d = nc.dram_tensor("bt", [R, D_OUT], f32, kind="ExternalInput")
    y_d = nc.dram_tensor("y", [S, T, D_OUT], f16, kind="ExternalOutput")

    with tile.TileContext(nc) as tc:
        with (
            tc.tile_pool(name="w", bufs=2) as wp,
            tc.tile_pool(name="bias", bufs=2) as bp,
            tc.tile_pool(name="x", bufs=3) as xp,
            tc.tile_pool(name="h", bufs=2) as hp,
            tc.tile_pool(name="y", bufs=4) as yp,
            tc.tile_pool(name="o", bufs=4) as op_,
            tc.tile_pool(name="st", bufs=8) as st,
            tc.tile_pool(name="c", bufs=1) as cp,
            tc.tile_pool(name="pro", bufs=1) as pro,
            tc.tile_pool(name="psum", bufs=8, space="PSUM") as pp,
        ):
            # PE pre-warm: matmuls on a zeroed tile while the first real
            # operands are in flight, so the PE clock (HAM ramp, ~3us of
            # sustained activity for 2.4GHz) is at full rate when the real
            # matmuls start. Warm outputs land in the same PSUM banks the
            # real accumulators later reuse (same pool tag), which keeps
            # them alive through DCE.
            warm_t = cp.tile([P, P], store_dt, name="warm_t")
            nc.vector.memset(warm_t, 0.0)
            eps_t = cp.tile([P, 1], f32)
            nc.vector.memset(eps_t, EPS)
            for w in range(N_WARM):
                warm_ps = pp.tile([P, T], f32, tag="ps", name=f"warm_ps_{w}")
                nc.tensor.matmul(
                    warm_ps[:, :P], lhsT=warm_t, rhs=warm_t, start=True, stop=True
                )

            xt_tiles: dict = {}
            slot = 0
            for r, L in enumerate(profile):
                # ---- per-run weight loads (double-buffered pool) ----
                if r == 0:
                    # prologue: per-k-chunk w1/xt loads so chunk k is usable
                    # as soon as it lands; the first matmul waits ~380KB.
                    # All on SP: the transfer queue (~1.1us per chunk pair)
                    # paces this anyway, and keeping ACT's queue free lets
                    # sample 0's ReLU copybacks start the moment PSUM is
                    # ready.
                    w1_ck, xt_ck = [], []
                    for k in range(KD):
                        wck = pro.tile([P, D_HID], store_dt, name=f"w1c{k}")
                        xck = pro.tile([P, T], store_dt, name=f"xtc{k}")
                        nc.sync.dma_start(out=wck, in_=w1_d[0, :, k, :])
                        nc.sync.dma_start(out=xck, in_=xt_d[0, :, k, :])
                        w1_ck.append(wck)
                        xt_ck.append(xck)

                    def w1_slice(k, h, _ck=w1_ck):
                        return _ck[k][:, P * h : P * (h + 1)]
                else:
                    w1_t = wp.tile([P, KD, D_HID], store_dt, tag="w1")
                    nc.sync.dma_start(out=w1_t, in_=w1_d[r])

                    def w1_slice(k, h, _t=w1_t):
                        return _t[:, k, P * h : P * (h + 1)]
                b1_t = bp.tile([P, KH], f32, tag="b1")
                nc.sync.dma_start(out=b1_t, in_=b1_d[r])
                w2_t = wp.tile([P, KH, D_OUT], store_dt, tag="w2")
                nc.sync.dma_start(out=w2_t[:, : KH // 2], in_=w2_d[r, :, : KH // 2])
                nc.sync.dma_start(out=w2_t[:, KH // 2 :], in_=w2_d[r, :, KH // 2 :])
                b2_t = bp.tile([P, 1, D_OUT], f32, tag="b2")
                nc.sync.dma_start(
                    out=b2_t, in_=b2_d[r : r + 1, :].partition_broadcast(P)
                )
                eb_t = bp.tile([P, 1], f32, tag="eb")
                nc.sync.dma_start(
                    out=eb_t, in_=eb_d[r : r + 1, :].partition_broadcast(P)
                )
                if apply_affine:
                    gm_t = bp.tile([P, 1, D_OUT], f32, tag="gm")
                    nc.sync.dma_start(
                        out=gm_t, in_=gm_d[r : r + 1, :].partition_broadcast(P)
                    )
                    bt_t = bp.tile([P, 1, D_OUT], f32, tag="bt")
                    nc.sync.dma_start(
                        out=bt_t, in_=bt_d[r : r + 1, :].partition_broadcast(P)
                    )

                for j in range(L):
                    s = slot
                    slot += 1
                    # prefetch next sample's activations (bufs=3 pool keeps
                    # the DMA well ahead of compute)
                    if s + 1 < S:
                        nxt = xp.tile([P, KD, T], store_dt, tag="xt")
                        nc.sync.dma_start(out=nxt, in_=xt_d[s + 1])
                        xt_tiles[s + 1] = nxt

                    # ---- pass 1: hT[h,:] = relu(W1[:,h].T @ xT + b1[h]) ----
                    hT_t = hp.tile([P, KH, T], store_dt, tag="hT")
                    if s == 0:
                        # k-outer over all 8 PSUM banks: matmuls start as
                        # soon as chunk k=0 has landed
                        ps_list = [
                            pp.tile([P, T], f32, tag="ps", name=f"ps0_{h}")
                            for h in range(KH)
                        ]
                        for k in range(KD):
                            for h in range(KH):
                                nc.tensor.matmul(
                                    ps_list[h],
                                    lhsT=w1_ck[k][:, P * h : P * (h + 1)],
                                    rhs=xt_ck[k],
                                    start=(k == 0),
                                    stop=(k == KD - 1),
                                )
                        for h in range(KH):
                            nc.scalar.activation(
                                out=hT_t[:, h, :],
                                in_=ps_list[h],
                                func=mybir.ActivationFunctionType.Relu,
                                bias=b1_t[:, h : h + 1],
                                scale=1.0,
                            )
                    else:
                        xt_t = xt_tiles.pop(s)
                        for h in range(KH):
                            ps = pp.tile([P, T], f32, tag="ps")
                            for k in range(KD):
                                nc.tensor.matmul(
                                    ps,
                                    lhsT=w1_slice(k, h),
                                    rhs=xt_t[:, k, :],
                                    start=(k == 0),
                                    stop=(k == KD - 1),
                                )
                            nc.scalar.activation(
                                out=hT_t[:, h, :],
                                in_=ps,
                                func=mybir.ActivationFunctionType.Relu,
                                bias=b1_t[:, h : h + 1],
                                scale=1.0,
                            )

                    # ---- pass 2 + LayerNorm ----
                    for t in range(MT):
                        tail_tile = s == S - 1 and t == MT - 1
                        ps2 = pp.tile([P, D_OUT], f32, tag="ps")
                        for k in range(KH):
                            if tail_tile and k == KH - 1:
                                # split the very last accumulation step by
                                # column halves: the first half of PSUM is
                                # final one matmul earlier, so the tail
                                # tile's stats start before PE even finishes
                                for hb, he in ((0, D_OUT // 2), (D_OUT // 2, D_OUT)):
                                    nc.tensor.matmul(
                                        ps2[:, hb:he],
                                        lhsT=hT_t[:, k, P * t : P * (t + 1)],
                                        rhs=w2_t[:, k, hb:he],
                                        start=False,
                                        stop=True,
                                    )
                                continue
                            nc.tensor.matmul(
                                ps2,
                                lhsT=hT_t[:, k, P * t : P * (t + 1)],
                                rhs=w2_t[:, k, :],
                                start=(k == 0),
                                stop=(k == KH - 1) and not tail_tile,
                            )
                        # The host passes c2 = b2 - mean(b2) in the b2
                        # slot (LN is shift-invariant, so this is exact).
                        # For the one tile whose epilogue is the exposed
                        # kernel tail, stats run DIRECTLY on PSUM (valid
                        # since mean(y+c2) = mean(y)) while the +c2 add runs
                        # in parallel on the idle Pool engine; var(c2) is
                        # folded into the sqrt bias (eb) and the dropped
                        # cov(y,c2) term is ~1e-3 relative. Everywhere else
                        # the exact add-then-stats DVE pipeline is kept.
                        y_t = yp.tile([P, D_OUT], f16, tag="y")
                        mv = st.tile([P, 2], f32, tag="mv")
                        if tail_tile:
                            # stats straight off PSUM (GPSIMD may not touch
                            # PSUM), one bn_stats per column half so the
                            # first starts before the last matmul retires;
                            # the +c2 add stays on DVE, emitted AFTER aggr
                            # so the ACT sqrt overlaps the add
                            stats2 = st.tile([P, 2, 6], f32, tag="stats2")
                            nc.vector.bn_stats(
                                out=stats2[:, 0, :], in_=ps2[:, : D_OUT // 2]
                            )
                            nc.vector.bn_stats(
                                out=stats2[:, 1, :], in_=ps2[:, D_OUT // 2 :]
                            )
                            nc.vector.bn_aggr(out=mv, in_=stats2)
                            nc.vector.tensor_add(out=y_t, in0=ps2, in1=b2_t[:, 0, :])
                        else:
                            stats = st.tile([P, 6], f32, tag="stats")
                            nc.vector.tensor_add(out=y_t, in0=ps2, in1=b2_t[:, 0, :])
                            nc.vector.bn_stats(out=stats, in_=y_t)
                            nc.vector.bn_aggr(out=mv, in_=stats)
                        rstd = st.tile([P, 1], f32, tag="rstd")
                        nc.scalar.activation(
                            out=rstd,
                            in_=mv[:, 1:2],
                            func=mybir.ActivationFunctionType.Sqrt,
                            bias=eb_t if tail_tile else eps_t,
                            scale=1.0,
                        )
                        nc.vector.reciprocal(out=rstd, in_=rstd)
                        o_t = op_.tile([P, D_OUT], f16, tag="o")
                        # for the last sample's earlier tiles, normalize on
                        # the idle Pool engine (SBUF-only, GPSIMD-legal) so
                        # the DVE queue is clear for the tail tile's chain
                        ts_eng = (
                            nc.gpsimd if (s == S - 1 and t < MT - 1) else nc.vector
                        )
                        ts_eng.tensor_scalar(
                            out=o_t,
                            in0=y_t,
                            scalar1=mv[:, 0:1],
                            scalar2=rstd,
                            op0=mybir.AluOpType.subtract,
                            op1=mybir.AluOpType.mult,
                        )
                        if apply_affine:
                            nc.vector.tensor_mul(out=o_t, in0=o_t, in1=gm_t[:, 0, :])
                            nc.vector.tensor_add(out=o_t, in0=o_t, in1=bt_t[:, 0, :])
                        nc.sync.dma_start(
                            out=y_d[s, P * t : P * (t + 1), :], in_=o_t
                        )
    nc.finalize()
    return nc


# --------------------------------------------------------------------------
# Host wrapper
# --------------------------------------------------------------------------

def kernel(**inputs) -> np.ndarray:
    global last_run_result
    x = np.asarray(inputs["x"], dtype=np.float32)
    day = np.asarray(inputs["day_indices"]).astype(np.int64)
    W1 = np.asarray(inputs["W1"], dtype=np.float32)
    b1 = np.asarray(inputs["b1"], dtype=np.float32)
    W2 = np.asarray(inputs["W2"], dtype=np.float32)
    b2 = np.asarray(inputs["b2"], dtype=np.float32)
    gamma = np.asarray(inputs["gamma"], dtype=np.float32)
    beta = np.asarray(inputs["beta"], dtype=np.float32)

    profile, runs_per_core, order = _assign(day)
    apply_affine = not (np.all(gamma == 1.0) and np.all(beta == 0.0))
    key = (profile, MM_DTYPE, apply_affine)
    if key not in _cache:
        _cache[key] = _build(*key)
    nc = _cache[key]

    mm_np = {
        "bfloat16": ml_dtypes.bfloat16,
        "float16": np.float16,
    }.get(MM_DTYPE, np.float32)

    # layout prep: contraction dim on partitions, partition-major so each
    # partition's DMA data is one contiguous DRAM run
    xt_all = np.ascontiguousarray(
        x.transpose(0, 2, 1).reshape(B, KD, P, T).transpose(0, 2, 1, 3).astype(mm_np)
    )
    nd = W1.shape[0]
    W1t = np.ascontiguousarray(
        W1.reshape(nd, KD, P, D_HID).transpose(0, 2, 1, 3).astype(mm_np)
    )
    W2t = np.ascontiguousarray(
        W2.reshape(nd, KH, P, D_OUT).transpose(0, 2, 1, 3).astype(mm_np)
    )
    b1t = np.ascontiguousarray(b1.reshape(nd, KH, P).transpose(0, 2, 1))
    c2 = b2 - b2.mean(axis=1, keepdims=True)
    eb = (EPS + c2.var(axis=1)).reshape(nd, 1).astype(np.float32)

    in_maps = []
    for c in range(N_CORES):
        day_list = [d for d, _ in runs_per_core[c]]
        ids = order[c]
        m = {
            "xt": np.ascontiguousarray(xt_all[ids]),
            "w1": np.ascontiguousarray(W1t[day_list]),
            "b1": np.ascontiguousarray(b1t[day_list]),
            "w2": np.ascontiguousarray(W2t[day_list]),
            "b2": np.ascontiguousarray(c2[day_list]),
            "eb": np.ascontiguousarray(eb[day_list]),
        }
        if apply_affine:
            m["gm"] = np.ascontiguousarray(gamma[day_list])
            m["bt"] = np.ascontiguousarray(beta[day_list])
        in_maps.append(m)

    trace = os.environ.get("DAYMLP_TRACE", "0") == "1"
    res = run_bass_kernel_spmd(
        nc,
        in_maps,
        core_ids=list(range(N_CORES)),
        trace=trace,
    )
    last_run_result = res
    out = np.empty((B, T, D_OUT), dtype=np.float32)
    for c in range(N_CORES):
        out[order[c]] = res.results[c]["y"].astype(np.float32)
    return out
